# revision 1
# baseline (speedup 1.0000x reference)
"""Multi-head attention (B=4, N=2048, DIM=1024, H=16, DH=64) on 8 trn2 cores.

Sharding: data-parallel over batch (4) x tensor-parallel over heads (2 groups
of 8). Each core computes q/k/v projections for its 8 heads, attention, and a
partial output projection; the host sums the two partials per batch and adds
the bias.

Layout (per core):
  - x^T produced on-chip via PE transposes; q^T/k^T kept transposed
    [inner, tok] so scores^T = k^T_tile.T @ q^T (contract DH=64) needs no
    transposes; v natural [tok, inner] with an appended ones column so
    attn@v (out^T = v_aug.T @ exp^T) yields softmax denominators for free
    in row 64; exp on ScalarE with 1/sqrt(dh) folded into the activation
    scale (max-subtraction skipped: |scores| < ~5 for this distribution).
    Normalization = DVE reciprocal + gpsimd partition-broadcast + DVE mult;
    the normalized transposed output feeds the Wo matmul directly as lhsT.
  - All matmuls in float32r (~1.5e-4 rel err, 4x faster than fp32).

Schedule (engine queues execute in program order, so emission order is the
schedule):
  - Phase A pipelines token-block tb's PE transposes (DMA-paced) with
    tb-1's projection matmul groups. The last block's q-projection is
    deferred into phase B as spread single-matmul filler.
  - Phase B emits per (head, query-block) "units": 2 score matmuls + exp +
    the same head's attn@v pair lagged 3 units + at most one filler matmul
    (previous block's Wo projection, or deferred q-projection). ScalarE
    receives a new score group every ~1.1us and stays saturated; the PE
    fills the rest of each unit with exp-independent work.
"""
import numpy as np

import concourse.bass as bass
import concourse.mybir as mybir
import concourse.tile as tile
from concourse import bacc
from concourse.bass_utils import run_bass_kernel_spmd
from concourse.masks import make_identity

f32 = mybir.dt.float32
f32r = mybir.dt.float32r
AF = mybir.ActivationFunctionType

N = 2048          # tokens
DIM = 1024        # model dim
NHL = 8           # heads per core
DH = 64           # head dim
INNER = NHL * DH  # 512 per-core inner dim
SCALE = DH ** -0.5
TB = 512          # token block (phase A)
QB = 512          # query block (phase B)
NTB = N // TB     # 4
NQB = N // QB     # 4
NKT = N // 128    # 16 k-tiles
NDC = DIM // 128  # 8 dim chunks
NM = INNER // 128 # 4 inner chunks
NG = NKT // 2     # 8 kt-pair groups per block

OPTS = dict(
    ps_s_bufs=2,
    big_bufs=16,
    wring_bufs=7,
    attnp_bufs=2,
    smallp_bufs=1,
    xin_bufs=2,
    outp_bufs=2,
    av_lag=3,
    defer_q3=True,
    take_pat=(3, 3, 3, 3),
)


def build_nc(**over):
    o = dict(OPTS)
    o.update(over)

    nc = bacc.Bacc(None, target_bir_lowering=False)

    x_d = nc.dram_tensor("x", [N, DIM], f32, kind="ExternalInput")
    wq_d = nc.dram_tensor("wq", [DIM, INNER], f32r, kind="ExternalInput")
    wk_d = nc.dram_tensor("wk", [DIM, INNER], f32r, kind="ExternalInput")
    wv_d = nc.dram_tensor("wv", [DIM, INNER], f32r, kind="ExternalInput")
    wo_d = nc.dram_tensor("wo", [INNER, DIM], f32r, kind="ExternalInput")
    out_d = nc.dram_tensor("out", [N, DIM], f32, kind="ExternalOutput")

    wq_v = wq_d.rearrange("(c k) n -> k c n", k=128)
    wk_v = wk_d.rearrange("(c k) n -> k c n", k=128)
    wv_v = wv_d.rearrange("(c k) n -> k c n", k=128)
    wo_v = wo_d.rearrange("(c k) n -> k c n", k=128)

    with tile.TileContext(nc) as tc:
        with (
            tc.tile_pool(name="consts", bufs=1) as consts,
            tc.tile_pool(name="xin", bufs=o["xin_bufs"]) as xin,
            tc.tile_pool(name="wring", bufs=o["wring_bufs"]) as wring,
            tc.tile_pool(name="wop", bufs=1) as wop,
            tc.tile_pool(name="big", bufs=o["big_bufs"]) as big,
            tc.tile_pool(name="ktp", bufs=1) as ktp,
            tc.tile_pool(name="vp", bufs=1) as vp,
            tc.tile_pool(name="attnp", bufs=o["attnp_bufs"]) as attnp,
            tc.tile_pool(name="outp", bufs=o["outp_bufs"]) as outp,
            tc.tile_pool(name="smallp", bufs=o["smallp_bufs"]) as smallp,
            tc.tile_pool(name="ps_s", bufs=o["ps_s_bufs"], space="PSUM") as ps_s,
            tc.tile_pool(name="ps_o", bufs=2, space="PSUM") as ps_o,
            tc.tile_pool(name="ps_f", bufs=2, space="PSUM") as ps_f,
        ):
            ident = consts.tile([128, 128], f32)
            make_identity(nc, ident)

            kT = ktp.tile([128, NM, N], f32r)              # k^T [inner, tok]
            v_sb = vp.tile([128, NKT, NHL, DH + 1], f32r)  # v + ones col

            ones_sb = consts.tile([128, NKT, NHL], f32)
            nc.vector.memset(ones_sb, 1.0)
            nc.vector.tensor_copy(v_sb[:, :, :, DH], ones_sb)

            wo_sb = wop.tile([128, NM, DIM], f32r)

            # ---------------- Phase A ----------------
            qT_slots = {}

            def emit_transpose_unit(tb, ts, xT):
                x_sub = xin.tile([128, DIM], f32, name=f"x{tb}{ts}",
                                 tag="xin")
                r0 = tb * TB + ts * 128
                if tb == 0 and ts == 0:
                    # split the very first load per column chunk so the
                    # first transposes start as soon as 64KB lands
                    for dc in range(NDC):
                        nc.sync.dma_start(
                            x_sub[:, dc * 128:dc * 128 + 128],
                            x_d[r0:r0 + 128, dc * 128:dc * 128 + 128])
                else:
                    nc.sync.dma_start(x_sub, x_d[r0:r0 + 128, :])
                for dc in range(NDC):
                    pt = ps_o.tile([128, 128], f32, name=f"pt{dc}", tag="o")
                    nc.tensor.transpose(
                        pt, x_sub[:, dc * 128:dc * 128 + 128], ident)
                    nc.vector.tensor_copy(
                        xT[dc // 2][:, dc % 2, ts * 128:ts * 128 + 128], pt)

            def q_mms(tb, xT, m):
                """Thunks: 8 matmuls of one q^T group + evac on the last."""
                psq = ps_f.tile([128, TB], f32, name=f"psq{tb}{m}", tag="f")
                wq_s = wq_state[tb]

                def mm(dc):
                    nc.tensor.matmul(
                        psq,
                        wq_s[dc // 2][:, dc % 2, m * 128:m * 128 + 128],
                        xT[dc // 2][:, dc % 2, :],
                        start=(dc == 0), stop=(dc == NDC - 1))
                    if dc == NDC - 1:
                        jm = m // 2
                        if (tb, jm) not in qT_slots:
                            qT_slots[(tb, jm)] = big.tile(
                                [128, 2, QB], f32r, name=f"qT{tb}{jm}",
                                tag="big")
                        nc.vector.tensor_copy(
                            qT_slots[(tb, jm)][:, m % 2, :], psq)
                return [(lambda dc=dc: mm(dc)) for dc in range(NDC)]

            wq_state = {}
            v_state = {}

            def proj_groups(tb, xT, include_q, include_v=True):
                """Generator of group-emitting thunks (k, v[, q] order)."""
                wk_s = [wring.tile([128, 2, INNER], f32r, name=f"wk{tb}{j}",
                                   tag="wr") for j in range(NDC // 2)]
                for j in range(NDC // 2):
                    nc.sync.dma_start(wk_s[j], wk_v[:, 2 * j:2 * j + 2, :])

                def k_group(m):
                    psk = ps_f.tile([128, TB], f32, name=f"psk{m}", tag="f")
                    for dc in range(NDC):
                        nc.tensor.matmul(
                            psk,
                            wk_s[dc // 2][:, dc % 2, m * 128:m * 128 + 128],
                            xT[dc // 2][:, dc % 2, :],
                            start=(dc == 0), stop=(dc == NDC - 1))
                    nc.vector.tensor_copy(kT[:, m, tb * TB:tb * TB + TB],
                                          psk)

                for m in range(NM):
                    yield (lambda m=m: k_group(m))

                wv_s = [wring.tile([128, 2, INNER], f32r, name=f"wv{tb}{j}",
                                   tag="wr") for j in range(NDC // 2)]
                for j in range(NDC // 2):
                    nc.sync.dma_start(wv_s[j], wv_v[:, 2 * j:2 * j + 2, :])

                def v_mms(ts):
                    psv = ps_f.tile([128, TB], f32, name=f"psv{tb}{ts}",
                                    tag="f")

                    def mm(dc):
                        nc.tensor.matmul(
                            psv,
                            xT[dc // 2][:, dc % 2, ts * 128:ts * 128 + 128],
                            wv_s[dc // 2][:, dc % 2, :],
                            start=(dc == 0), stop=(dc == NDC - 1))
                        if dc == NDC - 1:
                            kt = tb * (TB // 128) + ts
                            nc.vector.tensor_copy(
                                v_sb[:, kt, :, 0:DH],
                                psv.rearrange("p (h d) -> p h d", h=NHL))
                    return [(lambda dc=dc: mm(dc)) for dc in range(NDC)]

                v_state[tb] = v_mms
                if include_v:
                    for ts in range(TB // 128):
                        yield (lambda ts=ts: [t() for t in v_mms(ts)])

                wq_s = [wring.tile([128, 2, INNER], f32r, name=f"wq{tb}{j}",
                                   tag="wr") for j in range(NDC // 2)]
                for j in range(NDC // 2):
                    nc.sync.dma_start(wq_s[j], wq_v[:, 2 * j:2 * j + 2, :])
                wq_state[tb] = wq_s
                if include_q:
                    for m in range(NM):
                        yield (lambda m=m: [t() for t in q_mms(tb, xT, m)])

            xTs = {}
            prev_groups = None
            for tb in range(NTB):
                xT = [big.tile([128, 2, TB], f32r, name=f"xT{tb}{j}",
                               tag="big") for j in range(NDC // 2)]
                xTs[tb] = xT
                for ts in range(TB // 128):
                    emit_transpose_unit(tb, ts, xT)
                    if prev_groups is not None:
                        for _ in range(o["take_pat"][ts]):
                            next(prev_groups)()
                last = tb == NTB - 1
                defer = last and o["defer_q3"]
                prev_groups = proj_groups(tb, xT, include_q=not defer,
                                          include_v=True)
                if last:
                    for g in prev_groups:
                        g()

            nc.sync.dma_start(wo_sb, wo_v)

            # filler: single-matmul thunks consumed one per unit in phase B
            filler = []
            if o["defer_q3"]:
                for m in range(NM):
                    filler.extend(q_mms(NTB - 1, xTs[NTB - 1], m))

            # ---------------- Phase B ----------------
            def wo_unit_mms(qb, attnT, u):
                qs, d = u // 2, u % 2
                psf = ps_f.tile([128, 512], f32, name=f"psf{qs}{d}",
                                tag="f")

                def mm(m):
                    nc.tensor.matmul(
                        psf,
                        attnT[:, m, qs * 128:qs * 128 + 128],
                        wo_sb[:, m, d * 512:d * 512 + 512],
                        start=(m == 0), stop=(m == NM - 1))
                    if m == NM - 1:
                        osb = outp.tile([128, 512], f32, name=f"osb{qs}{d}",
                                        tag="osb")
                        nc.vector.tensor_copy(osb, psf)
                        r0 = qb * QB + qs * 128
                        nc.sync.dma_start(
                            out_d[r0:r0 + 128, d * 512:d * 512 + 512], osb)
                return [(lambda m=m: mm(m)) for m in range(NM)]

            def emit_av(h, pso, expT, g):
                for i in range(2):
                    kt = 2 * g + i
                    nc.tensor.matmul(
                        pso, v_sb[:, kt, h, :], expT[g][:, i, :],
                        start=(kt == 0), stop=(kt == NKT - 1))

            def emit_norm(h, pso, attnT):
                po = h % 2 * 64
                recip = smallp.tile([1, QB], f32, name=f"recip{h}",
                                    tag="recip")
                nc.vector.reciprocal(recip, pso[DH:DH + 1, :])
                bcast = smallp.tile([64, QB], f32, name=f"bcast{h}",
                                    tag="bcast")
                nc.gpsimd.partition_broadcast(bcast, recip)
                nc.vector.tensor_mul(attnT[po:po + 64, h // 2, :],
                                     pso[0:DH, :], bcast)

            lag = o["av_lag"]
            av_q = []   # (h, pso, et, g, is_last, attnT, qb)

            def deq():
                h, pso, et, g, is_last, attnT_, _qb = av_q.pop(0)
                for i in range(2):
                    kt = 2 * g + i
                    nc.tensor.matmul(
                        pso, v_sb[:, kt, h, :], et[:, i, :],
                        start=(kt == 0), stop=(kt == NKT - 1))
                if is_last:
                    emit_norm(h, pso, attnT_)

            for qb in range(NQB):
                attnT = attnp.tile([128, NM, QB], f32r, name=f"attnT{qb}",
                                   tag="attnT")
                for h in range(NHL):
                    po = h % 2 * 64
                    jm_q = (h // 2) // 2
                    im_q = (h // 2) % 2
                    qs_t = qT_slots[(qb, jm_q)]
                    pso = ps_o.tile([DH + 1, QB], f32, name=f"pso{h}",
                                    tag="o")
                    for g in range(NG):
                        pss = ps_s.tile([128, 2, QB], f32, name=f"pss{g}",
                                        tag="s")
                        for i in range(2):
                            kt = 2 * g + i
                            nc.tensor.matmul(
                                pss[:, i, :],
                                kT[po:po + 64, h // 2,
                                   kt * 128:kt * 128 + 128],
                                qs_t[po:po + 64, im_q, :],
                                start=True, stop=True)
                        et = big.tile([128, 2, QB], f32r, name=f"eT{h}{g}",
                                      tag="big")
                        nc.scalar.activation(out=et, in_=pss, func=AF.Exp,
                                             scale=SCALE)
                        av_q.append((h, pso, et, g, g == NG - 1, attnT, qb))
                        if len(av_q) > lag:
                            deq()
                        if filler:
                            filler.pop(0)()
                            if len(filler) > 32:
                                filler.pop(0)()

                if qb + 1 < NQB:
                    for u in range(8):
                        filler.extend(wo_unit_mms(qb, attnT, u))

            while av_q:
                deq()
            for u in range(8):
                for t in wo_unit_mms(NQB - 1, attnT, u):
                    t()

    nc.compile()
    return nc


_NC = None


def _get_nc():
    global _NC
    if _NC is None:
        _NC = build_nc()
    return _NC


def kernel(x, Wq, Wk, Wv, Wo, bo):
    x = np.ascontiguousarray(np.asarray(x, dtype=np.float32))
    Wq = np.asarray(Wq, dtype=np.float32)
    Wk = np.asarray(Wk, dtype=np.float32)
    Wv = np.asarray(Wv, dtype=np.float32)
    Wo = np.asarray(Wo, dtype=np.float32)
    bo = np.asarray(bo, dtype=np.float32)

    B = x.shape[0]
    nc = _get_nc()
    in_maps = []
    for c in range(8):
        b, hh = c // 2, c % 2
        sl = slice(hh * INNER, hh * INNER + INNER)
        in_maps.append({
            "x": np.ascontiguousarray(x[b]),
            "wq": np.ascontiguousarray(Wq[:, sl]),
            "wk": np.ascontiguousarray(Wk[:, sl]),
            "wv": np.ascontiguousarray(Wv[:, sl]),
            "wo": np.ascontiguousarray(Wo[sl, :]),
        })
    res = run_bass_kernel_spmd(nc, in_maps, core_ids=list(range(8)))
    out = np.empty((B, N, DIM), dtype=np.float32)
    for b in range(B):
        out[b] = res.results[2 * b]["out"] + res.results[2 * b + 1]["out"] + bo
    return out



# revision 9
# speedup vs baseline: 1.0002x; 1.0002x over previous
"""Multi-head attention (B=4, N=2048, DIM=1024, H=16, DH=64) on 8 trn2 cores.

Sharding: data-parallel over batch (4) x tensor-parallel over heads (2 groups
of 8). Each core computes q/k/v projections for its 8 heads, attention, and a
partial output projection; the host sums the two partials per batch, applies
1/16 scale compensation is done in-kernel, and adds the bias.

Design (per core):
  - Host sends x pre-transposed (xT [DIM, N] f32) - no on-chip transposes.
  - Projections in f32r (exact): psum = q/k/v unscaled.
  - q/k evacuate to fp8e4 at scale 16 in a [32-row x 2-slot] per-head layout
    (host permutes Wq/Wk columns) so the score matmuls run in fp8 DoubleRow
    mode (2 k-tiles per instruction, 0.5 cyc/row): scores_psum = 256*q.k.
  - v evacuates to fp8 hi + fp8 lo residual (scalar_tensor_tensor), giving
    ~fp16 effective precision for attn@v while keeping DoubleRow speed.
  - exp: split across ScalarE (native Exp -> fp8, scale 1/2048 folded) and
    DVE/GPSIMD (Schraudolph: i8 = round(psum*A + B) written to an int8 view
    of the fp8 tile - the bitcast i8 IS the fp8 exp approximation).
  - attn@v flipped: out[q, dh] = expT.T @ v_aug per query-tile, DoubleRow
    over kt pairs; ones column (C=1) in v_hi gives the softmax denominator
    per PARTITION (query), so normalization is a cheap per-partition
    reciprocal + broadcast multiply writing fp16 attn.
  - attn fp16 is PE-transposed to attnT [inner, tok] (psum fp16), then the
    output projection runs fp16 x fp16 (Wo fp16 from host), psum = 16*out;
    evacuation scales by 1/16.
"""
import numpy as np
import ml_dtypes

import concourse.bass as bass
import concourse.mybir as mybir
import concourse.tile as tile
from concourse import bacc
from concourse.bass_utils import run_bass_kernel_spmd
from concourse.masks import make_identity

f32 = mybir.dt.float32
f32r = mybir.dt.float32r
f16 = mybir.dt.float16
f8 = mybir.dt.float8e4
i8 = mybir.dt.int8
AF = mybir.ActivationFunctionType
ALU = mybir.AluOpType
DR = mybir.MatmulPerfMode.DoubleRow
FP8 = ml_dtypes.float8_e4m3

N = 2048          # tokens
DIM = 1024        # model dim
NHL = 8           # heads per core
DH = 64           # head dim
INNER = NHL * DH  # 512 per-core inner dim
SCALE = DH ** -0.5
NDC = DIM // 128  # 8 dim chunks
SQ = 16.0         # q/k/v fp8 scale
EXP_A = 8 * 1.4426950408889634 * SCALE / (SQ * SQ)   # i8 slope
EXP_B = 8 * (7.0 - 0.0579)                           # i8 bias
QB = 512
NQB = N // QB
NG = 8            # kt-pair groups per query block

# exp engine pattern per unit: alternate ScalarE / DVE (gpsimd cannot
# read PSUM on trn2, so it cannot help with exp or evacuations)
EXP_PAT = (0, 1, 0, 1, 0, 1, 0, 1)
AV_LAG = 4


def build_nc():
    nc = bacc.Bacc(None, target_bir_lowering=False)

    xT_d = nc.dram_tensor("xT", [DIM, N], f32r, kind="ExternalInput")
    wq_d = nc.dram_tensor("wq", [DIM, INNER], f32r, kind="ExternalInput")
    wk_d = nc.dram_tensor("wk", [DIM, INNER], f32r, kind="ExternalInput")
    wv_d = nc.dram_tensor("wv", [DIM, INNER], f32r, kind="ExternalInput")
    wo_d = nc.dram_tensor("wo", [INNER, DIM], f16, kind="ExternalInput")
    out_d = nc.dram_tensor("out", [N, DIM], f32, kind="ExternalOutput")

    xT_v = xT_d.rearrange("(c k) n -> k c n", k=128)   # [128, 8, N]
    wq_v = wq_d.rearrange("(c k) n -> k c n", k=128)   # [128, 8, INNER]
    wk_v = wk_d.rearrange("(c k) n -> k c n", k=128)
    wv_v = wv_d.rearrange("(c k) n -> k c n", k=128)
    wo_v = wo_d.rearrange("(c k) n -> k c n", k=128)   # [128, 4, DIM]

    with tile.TileContext(nc) as tc:
        with (
            tc.tile_pool(name="consts", bufs=1) as consts,
            tc.tile_pool(name="xin", bufs=1) as xin,
            tc.tile_pool(name="wp", bufs=1) as wp,
            tc.tile_pool(name="wop", bufs=1) as wop,
            tc.tile_pool(name="qk", bufs=1) as qkp,
            tc.tile_pool(name="vp", bufs=1) as vp,
            tc.tile_pool(name="expp", bufs=AV_LAG + 3) as expp,
            tc.tile_pool(name="anat", bufs=2) as anatp,
            tc.tile_pool(name="atp", bufs=2) as atp,
            tc.tile_pool(name="outp", bufs=2) as outp,
            tc.tile_pool(name="recipp", bufs=3) as recipp,
            tc.tile_pool(name="ps_s", bufs=2, space="PSUM") as ps_s,
            tc.tile_pool(name="ps_av", bufs=2, space="PSUM") as ps_av,
            tc.tile_pool(name="ps_f", bufs=1, space="PSUM") as ps_f,
        ):
            ident = consts.tile([128, 128], f16)
            make_identity(nc, ident)

            # --- input DMAs (order matters: k weights + x chunks first)
            wk_sb = wp.tile([128, NDC, INNER], f32r)
            wv_sb = wp.tile([128, NDC, INNER], f32r)
            wq_sb = wp.tile([128, NDC, INNER], f32r)
            wo_sb = wop.tile([128, 4, DIM], f16)
            xts = [xin.tile([128, 2, N], f32r, name=f"xt{j}", tag=f"xt{j}")
                   for j in range(4)]
            nc.sync.dma_start(wk_sb, wk_v[:, :, :])
            nc.sync.dma_start(xts[0], xT_v[:, 0:2, :])
            nc.sync.dma_start(xts[1], xT_v[:, 2:4, :])
            nc.sync.dma_start(wv_sb, wv_v[:, :, :])
            nc.sync.dma_start(xts[2], xT_v[:, 4:6, :])
            nc.sync.dma_start(xts[3], xT_v[:, 6:8, :])
            nc.sync.dma_start(wq_sb, wq_v[:, :, :])
            nc.sync.dma_start(wo_sb, wo_v[:, :, :])

            # fp8 q/k in 32x2 DoubleRow layout: [128, j, pair, tok]
            kT2 = qkp.tile([128, 2, 2, N], f8)
            qT2 = qkp.tile([128, 2, 2, N], f8)
            # v hi/lo: [key%128, ktpair, i, h, DH+1]
            v_hi = vp.tile([128, NG, 2, NHL, DH + 1], f8)
            v_lo = vp.tile([128, NG, 2, NHL, DH + 1], f8)
            nc.vector.memset(v_hi[:, :, :, :, DH], 1.0)
            nc.vector.memset(v_lo[:, :, :, :, DH], 0.0)

            # ---------------- Phase A: projections (f32r) ----------------
            evac_rr = [0]

            def qk_proj(w_sb, dest, tb):
                t0 = tb * 512
                for j in range(2):
                    ps = ps_s.tile([128, 2, 512], f32, name=f"pp{j}", tag="s")
                    for p in range(2):
                        m0 = (2 * j + p) * 128
                        for dc in range(NDC):
                            nc.tensor.matmul(
                                ps[:, p, :],
                                w_sb[:, dc, m0:m0 + 128],
                                xts[dc // 2][:, dc % 2, t0:t0 + 512],
                                start=(dc == 0), stop=(dc == NDC - 1))
                    if evac_rr[0] % 2 == 0:
                        nc.vector.tensor_scalar(dest[:, j, :, t0:t0 + 512],
                                                ps, SQ, None, ALU.mult)
                    else:
                        nc.scalar.activation(out=dest[:, j, :, t0:t0 + 512],
                                             in_=ps, func=AF.Copy, scale=SQ)
                    evac_rr[0] += 1

            def v_proj(tb):
                for jj in range(2):
                    g = tb * 2 + jj
                    ps = ps_f.tile([128, 2, 512], f32, name=f"pv{jj}",
                                   tag="f")
                    for i in range(2):
                        kt = 4 * tb + 2 * jj + i
                        for dc in range(NDC):
                            nc.tensor.matmul(
                                ps[:, i, :],
                                xts[dc // 2][:, dc % 2,
                                             kt * 128:kt * 128 + 128],
                                wv_sb[:, dc, :],
                                start=(dc == 0), stop=(dc == NDC - 1))
                    psv = ps.rearrange("p two (h d) -> p two h d", h=NHL)
                    nc.scalar.activation(out=v_hi[:, g, :, :, 0:DH],
                                          in_=psv, func=AF.Copy, scale=SQ)
                    nc.vector.scalar_tensor_tensor(
                        v_lo[:, g, :, :, 0:DH], psv, SQ,
                        v_hi[:, g, :, :, 0:DH], ALU.mult, ALU.subtract)

            for tb in range(4):
                qk_proj(wk_sb, kT2, tb)
                v_proj(tb)
                qk_proj(wq_sb, qT2, tb)

            # ---------------- Phase B: attention ----------------
            av_q = []  # (av, et, g, h, is_last)

            def deq():
                av, et, g, h, is_last = av_q.pop(0)
                # one accumulation group for the whole bank: start only on
                # the very first matmul, stop on the very last (psum zero
                # regions are 2KB, so per-qt groups would wipe each other)
                for qt in range(4):
                    lhs = et[:, :, qt * 128:qt * 128 + 128]
                    nc.tensor.matmul(av[:, qt, :], lhs, v_hi[:, g, :, h, :],
                                     start=(g == 0 and qt == 0), stop=False,
                                     perf_mode=DR, skip_group_check=True)
                    nc.tensor.matmul(av[:, qt, :], lhs, v_lo[:, g, :, h, :],
                                     start=False,
                                     stop=(g == NG - 1 and qt == 3),
                                     perf_mode=DR, skip_group_check=True)

            def emit_norm(av, anat, h):
                recip = recipp.tile([128, 4, 1], f32, name=f"rc{h}",
                                    tag="rc")
                nc.vector.reciprocal(recip, av[:, :, DH:DH + 1])
                nc.vector.tensor_tensor(
                    anat[:, :, h, :], av[:, :, 0:DH],
                    recip.broadcast_to([128, 4, DH]), ALU.mult)

            def wo_unit(anat, qb):
                # transposes: attnT[inner, tok] from anat [q, inner]
                attnT = atp.tile([128, 4, QB], f16, name=f"aT{qb}",
                                 tag="aT")
                for qt in range(4):
                    ptr = ps_av.tile([128, 4, 128], f16, name=f"ptr{qt}",
                                     tag="av")
                    av_in = anat[:, qt, :, :].rearrange("p h d -> p (h d)")
                    for c in range(4):
                        nc.tensor.transpose(
                            ptr[:, c, :], av_in[:, c * 128:c * 128 + 128],
                            ident)
                    nc.scalar.activation(
                        out=attnT[:, :, qt * 128:qt * 128 + 128], in_=ptr,
                        func=AF.Copy, scale=1.0)
                for qs in range(4):
                    psf = ps_f.tile([128, 2, 512], f32, name=f"psf{qs}",
                                    tag="f")
                    for d in range(2):
                        for c in range(4):
                            nc.tensor.matmul(
                                psf[:, d, :],
                                attnT[:, c, qs * 128:qs * 128 + 128],
                                wo_sb[:, c, d * 512:d * 512 + 512],
                                start=(c == 0), stop=(c == 3))
                    osb = outp.tile([128, DIM], f32, name=f"osb{qs}",
                                    tag="osb")
                    nc.scalar.activation(out=osb, in_=psf, func=AF.Copy,
                                         scale=1.0 / SQ)
                    r0 = qb * QB + qs * 128
                    nc.sync.dma_start(out_d[r0:r0 + 128, :], osb)

            engs = (None, nc.vector, nc.gpsimd)
            for qb in range(NQB):
                anat = anatp.tile([128, 4, NHL, DH], f16, name=f"an{qb}",
                                  tag="an")
                for h in range(NHL):
                    j, po = h // 4, 32 * (h % 4)
                    av = ps_av.tile([128, 4, DH + 1], f32, name=f"av{h}",
                                    tag="av")
                    for g in range(NG):
                        pss = ps_s.tile([128, 2, QB], f32, name=f"pss{g}",
                                        tag="s")
                        for i in range(2):
                            kt = 2 * g + i
                            nc.tensor.matmul(
                                pss[:, i, :],
                                kT2[po:po + 32, j, :,
                                    kt * 128:kt * 128 + 128],
                                qT2[po:po + 32, j, :, qb * QB:(qb + 1) * QB],
                                start=True, stop=True, perf_mode=DR,
                                tile_position=(po, 0))
                        et = expp.tile([128, 2, QB], f8, name=f"et{h}{g}",
                                       tag="et")
                        w = EXP_PAT[(h + qb + g) % 8]
                        if w == 0:
                            nc.scalar.activation(out=et, in_=pss,
                                                 func=AF.Exp,
                                                 scale=SCALE / (SQ * SQ))
                        else:
                            engs[w].tensor_scalar(et.bitcast(i8), pss,
                                                  EXP_A, EXP_B,
                                                  ALU.mult, ALU.add)
                        av_q.append((av, et, g, h, g == NG - 1))
                        if len(av_q) > AV_LAG:
                            deq()
                    if h > 0:
                        emit_norm(prev_av, anat, h - 1)
                    prev_av = av
                while av_q:
                    deq()
                emit_norm(prev_av, anat, NHL - 1)
                wo_unit(anat, qb)

    nc.compile()
    return nc


_NC = None


def _get_nc():
    global _NC
    if _NC is None:
        _NC = build_nc()
    return _NC


def _perm():
    """Column permutation for Wq/Wk: new col (j*256 + p*128 + r) holds
    original inner dim h*64 + t with h = 4j + r//32, t = 32p + r%32."""
    perm = np.empty(INNER, dtype=np.int64)
    for j in range(2):
        for p in range(2):
            for r in range(128):
                h = 4 * j + r // 32
                t = 32 * p + r % 32
                perm[j * 256 + p * 128 + r] = h * DH + t
    return perm


def kernel(x, Wq, Wk, Wv, Wo, bo):
    x = np.asarray(x, dtype=np.float32)
    Wq = np.asarray(Wq, dtype=np.float32)
    Wk = np.asarray(Wk, dtype=np.float32)
    Wv = np.asarray(Wv, dtype=np.float32)
    Wo = np.asarray(Wo, dtype=np.float32)
    bo = np.asarray(bo, dtype=np.float32)

    B = x.shape[0]
    perm = _perm()
    nc = _get_nc()
    in_maps = []
    for c in range(8):
        b, hh = c // 2, c % 2
        sl = slice(hh * INNER, hh * INNER + INNER)
        in_maps.append({
            "xT": np.ascontiguousarray(x[b].T),
            "wq": np.ascontiguousarray(Wq[:, sl][:, perm]),
            "wk": np.ascontiguousarray(Wk[:, sl][:, perm]),
            "wv": np.ascontiguousarray(Wv[:, sl]),
            "wo": np.ascontiguousarray(Wo[sl, :].astype(np.float16)),
        })
    res = run_bass_kernel_spmd(nc, in_maps, core_ids=list(range(8)))
    out = np.empty((B, N, DIM), dtype=np.float32)
    for b in range(B):
        out[b] = res.results[2 * b]["out"] + res.results[2 * b + 1]["out"] + bo
    return out


# revision 12
# speedup vs baseline: 1.0306x; 1.0303x over previous
"""Multi-head attention (B=4, N=2048, DIM=1024, H=16, DH=64) on 8 trn2 cores.

Sharding: data-parallel over batch (4) x tensor-parallel over heads (2 groups
of 8). Each core computes q/k/v projections for its 8 heads, attention, and a
partial output projection; the host sums the two partials per batch and adds
the bias.

Design (per core):
  - Host sends x pre-transposed (xT [DIM, N] f32) - no on-chip transposes.
  - Projections in f32r (exact): psum = q/k/v unscaled.
  - q/k evacuate to fp8e4 at scale 16 in a [32-row x 2-slot] per-head layout
    (host permutes Wq/Wk columns) so the score matmuls run in fp8 DoubleRow
    mode (2 k-tiles per instruction, 0.5 cyc/row): scores_psum = 256*q.k.
  - v evacuates to fp8 hi + fp8 lo residual (scalar_tensor_tensor), giving
    ~fp16 effective precision for attn@v while keeping DoubleRow speed.
  - exp: split across ScalarE (native Exp -> fp8, scale 1/2048 folded) and
    DVE (Schraudolph: i8 = round(psum*A + B) written to an int8 view of the
    fp8 tile - the bitcast i8 IS the fp8 exp approximation). GPSIMD cannot
    read PSUM on trn2 so it only issues the x DMAs.
  - attn@v flipped: out[q, dh] = expT.T @ v_aug per query-tile, DoubleRow
    over kt pairs, one psum accumulation group per bank (2KB zero region);
    ones column (0.25) in v_hi gives the softmax denominator per PARTITION
    (query): normalization is reciprocal + stride-0-broadcast multiply
    writing fp16 attn.
  - attn fp16 is PE-transposed to attnT (psum fp16), then the output
    projection runs fp16 x fp16 (Wo fp16 from host), psum = 16*out, evac
    scales 1/16.
  - Schedule: short phase A (k + q(qb0) projections, DMA-paced on two
    queues), then one exp-bound main loop over (qb, h, g) units. v and
    q(qb1-3) projections and the previous qb's transposes + Wo matmuls are
    queued as filler thunks the PE consumes between score matmuls.
"""
import numpy as np
import ml_dtypes

import concourse.bass as bass
import concourse.mybir as mybir
import concourse.tile as tile
from concourse import bacc
from concourse.bass_utils import run_bass_kernel_spmd
from concourse.masks import make_identity

f32 = mybir.dt.float32
f32r = mybir.dt.float32r
f16 = mybir.dt.float16
f8 = mybir.dt.float8e4
i8 = mybir.dt.int8
AF = mybir.ActivationFunctionType
ALU = mybir.AluOpType
DR = mybir.MatmulPerfMode.DoubleRow
FP8 = ml_dtypes.float8_e4m3

N = 2048          # tokens
DIM = 1024        # model dim
NHL = 8           # heads per core
DH = 64           # head dim
INNER = NHL * DH  # 512 per-core inner dim
SCALE = DH ** -0.5
NDC = DIM // 128  # 8 dim chunks
SQ = 16.0         # q/k/v fp8 scale
EXP_A = 8 * 1.4426950408889634 * SCALE / (SQ * SQ)   # i8 slope
EXP_B = 8 * (7.0 - 0.0579)                           # i8 bias
QB = 512
NQB = N // QB
NG = 8            # kt-pair groups per query block

EXP_PAT = (0, 1, 0, 1, 0, 1, 0, 1)   # 0=ScalarE, 1=DVE
AV_LAG = 5
FILL_PER_G = 1


def build_nc():
    nc = bacc.Bacc(None, target_bir_lowering=False)

    xT_d = nc.dram_tensor("xT", [DIM, N], f32r, kind="ExternalInput")
    wq_d = nc.dram_tensor("wq", [DIM, INNER], f32r, kind="ExternalInput")
    wk_d = nc.dram_tensor("wk", [DIM, INNER], f32r, kind="ExternalInput")
    wv_d = nc.dram_tensor("wv", [DIM, INNER], f32r, kind="ExternalInput")
    wo_d = nc.dram_tensor("wo", [INNER, DIM], f16, kind="ExternalInput")
    out_d = nc.dram_tensor("out", [N, DIM], f32, kind="ExternalOutput")

    xT_v = xT_d.rearrange("(c k) n -> k c n", k=128)   # [128, 8, N]
    wq_v = wq_d.rearrange("(c k) n -> k c n", k=128)   # [128, 8, INNER]
    wk_v = wk_d.rearrange("(c k) n -> k c n", k=128)
    wv_v = wv_d.rearrange("(c k) n -> k c n", k=128)
    wo_v = wo_d.rearrange("(c k) n -> k c n", k=128)   # [128, 4, DIM]

    with tile.TileContext(nc) as tc:
        with (
            tc.tile_pool(name="consts", bufs=1) as consts,
            tc.tile_pool(name="xin", bufs=1) as xin,
            tc.tile_pool(name="wp", bufs=1) as wp,
            tc.tile_pool(name="wop", bufs=1) as wop,
            tc.tile_pool(name="qk", bufs=1) as qkp,
            tc.tile_pool(name="vp", bufs=1) as vp,
            tc.tile_pool(name="expp", bufs=AV_LAG + 3) as expp,
            tc.tile_pool(name="anat", bufs=2) as anatp,
            tc.tile_pool(name="atp", bufs=2) as atp,
            tc.tile_pool(name="outp", bufs=2) as outp,
            tc.tile_pool(name="recipp", bufs=3) as recipp,
            tc.tile_pool(name="ps_s", bufs=2, space="PSUM") as ps_s,
            tc.tile_pool(name="ps_av", bufs=2, space="PSUM") as ps_av,
            tc.tile_pool(name="ps_f", bufs=1, space="PSUM") as ps_f,
        ):
            ident = consts.tile([128, 128], f16)
            make_identity(nc, ident)

            wk_sb = wp.tile([128, NDC, INNER], f32r)
            wv_sb = wp.tile([128, NDC, INNER], f32r)
            wq_sb = wp.tile([128, NDC, INNER], f32r)
            wo_sb = wop.tile([128, 4, DIM], f16)
            xts = [xin.tile([128, 2, N], f32r, name=f"xt{j}", tag=f"xt{j}")
                   for j in range(4)]
            # weights on the SP DMA queue; x token-pieces on the Pool queue
            nc.sync.dma_start(wk_sb, wk_v[:, :, :])
            for tb in range(4):
                t0 = tb * 512
                for j in range(4):
                    nc.gpsimd.dma_start(
                        xts[j][:, :, t0:t0 + 512],
                        xT_v[:, 2 * j:2 * j + 2, t0:t0 + 512])
            nc.sync.dma_start(wq_sb, wq_v[:, :, :])
            nc.sync.dma_start(wv_sb, wv_v[:, :, :])
            nc.sync.dma_start(wo_sb, wo_v[:, :, :])

            # fp8 q/k in 32x2 DoubleRow layout: [128, j, pair, tok]
            kT2 = qkp.tile([128, 2, 2, N], f8)
            qT2 = qkp.tile([128, 2, 2, N], f8)
            # v hi/lo: [key%128, ktpair, i, h, DH+1]; ones col
            v_hi = vp.tile([128, NG, 2, NHL, DH + 1], f8)
            v_lo = vp.tile([128, NG, 2, NHL, DH + 1], f8)
            nc.vector.memset(v_hi[:, :, :, :, DH], 1.0)
            nc.vector.memset(v_lo[:, :, :, :, DH], 0.0)

            # ---------------- projection emitters ----------------
            evac_rr = [0]

            def qk_chain(w_sb, dest, tb, j):
                t0 = tb * 512
                ps = ps_s.tile([128, 2, 512], f32, name=f"pp{tb}{j}",
                               tag="s")
                for p in range(2):
                    m0 = (2 * j + p) * 128
                    for dc in range(NDC):
                        nc.tensor.matmul(
                            ps[:, p, :],
                            w_sb[:, dc, m0:m0 + 128],
                            xts[dc // 2][:, dc % 2, t0:t0 + 512],
                            start=(dc == 0), stop=(dc == NDC - 1))
                if evac_rr[0] % 2 == 0:
                    nc.vector.tensor_scalar(dest[:, j, :, t0:t0 + 512],
                                            ps, SQ, None, ALU.mult)
                else:
                    nc.scalar.activation(out=dest[:, j, :, t0:t0 + 512],
                                         in_=ps, func=AF.Copy, scale=SQ)
                evac_rr[0] += 1

            def v_chain(g):
                # one kt-pair g -> v_hi/v_lo tile pair
                ps = ps_f.tile([128, 2, 512], f32, name=f"pv{g}", tag="f")
                for i in range(2):
                    kt = 2 * g + i
                    for dc in range(NDC):
                        nc.tensor.matmul(
                            ps[:, i, :],
                            xts[dc // 2][:, dc % 2,
                                         kt * 128:kt * 128 + 128],
                            wv_sb[:, dc, :],
                            start=(dc == 0), stop=(dc == NDC - 1))
                psv = ps.rearrange("p two (h d) -> p two h d", h=NHL)
                nc.scalar.activation(out=v_hi[:, g, :, :, 0:DH],
                                     in_=psv, func=AF.Copy, scale=SQ)
                nc.vector.scalar_tensor_tensor(
                    v_lo[:, g, :, :, 0:DH], psv, SQ,
                    v_hi[:, g, :, :, 0:DH], ALU.mult, ALU.subtract)

            # ---------------- phase A: k (all) + q(qb0) ----------------
            for tb in range(4):
                for j in range(2):
                    qk_chain(wk_sb, kT2, tb, j)
            for j in range(2):
                qk_chain(wq_sb, qT2, 0, j)

            # filler: v chains first (needed by attn@v), then q(qb1-3)
            filler = []
            for g in range(NG):
                filler.append(lambda g=g: v_chain(g))
            for tb in range(1, 4):
                for j in range(2):
                    filler.append(
                        lambda tb=tb, j=j: qk_chain(wq_sb, qT2, tb, j))

            # ---------------- attention main loop ----------------
            av_q = []  # (av, et, g, h, is_last)

            def deq():
                av, et, g, h, is_last = av_q.pop(0)
                # one accumulation group for the whole bank: start only on
                # the very first matmul, stop on the very last (psum zero
                # regions are 2KB, so per-qt groups would wipe each other)
                for qt in range(4):
                    lhs = et[:, :, qt * 128:qt * 128 + 128]
                    nc.tensor.matmul(av[:, qt, :], lhs, v_hi[:, g, :, h, :],
                                     start=(g == 0 and qt == 0), stop=False,
                                     perf_mode=DR, skip_group_check=True)
                    nc.tensor.matmul(av[:, qt, :], lhs, v_lo[:, g, :, h, :],
                                     start=False,
                                     stop=(g == NG - 1 and qt == 3),
                                     perf_mode=DR, skip_group_check=True)

            def emit_norm(av, anat, h):
                recip = recipp.tile([128, 4, 1], f32, name=f"rc{h}",
                                    tag="rc")
                nc.vector.reciprocal(recip, av[:, :, DH:DH + 1])
                nc.vector.tensor_tensor(
                    anat[:, :, h, :], av[:, :, 0:DH],
                    recip.broadcast_to([128, 4, DH]), ALU.mult)

            def wo_fillers(anat, qb):
                """Filler thunks for one qb: 4 transpose quads + 4 Wo
                chains (each with evac + out DMA)."""
                attnT = atp.tile([128, 4, QB], f16, name=f"aT{qb}",
                                 tag="aT")

                def transp(qt):
                    ptr = ps_av.tile([128, 4, 128], f16, name=f"ptr{qt}",
                                     tag="av")
                    av_in = anat[:, qt, :, :].rearrange("p h d -> p (h d)")
                    for c in range(4):
                        nc.tensor.transpose(
                            ptr[:, c, :], av_in[:, c * 128:c * 128 + 128],
                            ident)
                    nc.scalar.activation(
                        out=attnT[:, :, qt * 128:qt * 128 + 128], in_=ptr,
                        func=AF.Copy, scale=1.0)

                def wo_chain(qs):
                    psf = ps_f.tile([128, 2, 512], f32, name=f"psf{qs}",
                                    tag="f")
                    for d in range(2):
                        for c in range(4):
                            nc.tensor.matmul(
                                psf[:, d, :],
                                attnT[:, c, qs * 128:qs * 128 + 128],
                                wo_sb[:, c, d * 512:d * 512 + 512],
                                start=(c == 0), stop=(c == 3))
                    osb = outp.tile([128, DIM], f32, name=f"osb{qs}",
                                    tag="osb")
                    nc.scalar.activation(out=osb, in_=psf, func=AF.Copy,
                                         scale=1.0 / SQ)
                    r0 = qb * QB + qs * 128
                    nc.sync.dma_start(out_d[r0:r0 + 128, :], osb)

                return ([(lambda qt=qt: transp(qt)) for qt in range(4)]
                        + [(lambda qs=qs: wo_chain(qs)) for qs in range(4)])

            for qb in range(NQB):
                anat = anatp.tile([128, 4, NHL, DH], f16, name=f"an{qb}",
                                  tag="an")
                for h in range(NHL):
                    j, po = h // 4, 32 * (h % 4)
                    av = ps_av.tile([128, 4, DH + 1], f32, name=f"av{h}",
                                    tag="av")
                    for g in range(NG):
                        pss = ps_s.tile([128, 2, QB], f32, name=f"pss{g}",
                                        tag="s")
                        for i in range(2):
                            kt = 2 * g + i
                            nc.tensor.matmul(
                                pss[:, i, :],
                                kT2[po:po + 32, j, :,
                                    kt * 128:kt * 128 + 128],
                                qT2[po:po + 32, j, :, qb * QB:(qb + 1) * QB],
                                start=True, stop=True, perf_mode=DR,
                                tile_position=(po, 0))
                        et = expp.tile([128, 2, QB], f8, name=f"et{h}{g}",
                                       tag="et")
                        if EXP_PAT[(h + qb + g) % 8] == 0:
                            nc.scalar.activation(out=et, in_=pss,
                                                 func=AF.Exp,
                                                 scale=SCALE / (SQ * SQ))
                        else:
                            nc.vector.tensor_scalar(et.bitcast(i8), pss,
                                                    EXP_A, EXP_B,
                                                    ALU.mult, ALU.add)
                        av_q.append((av, et, g, h, g == NG - 1))
                        if len(av_q) > AV_LAG:
                            deq()
                        for _ in range(FILL_PER_G):
                            if filler:
                                filler.pop(0)()
                    if h > 0:
                        emit_norm(prev_av, anat, h - 1)
                    prev_av = av
                while av_q:
                    deq()
                emit_norm(prev_av, anat, NHL - 1)
                filler.extend(wo_fillers(anat, qb))
            while filler:
                filler.pop(0)()

    nc.compile()
    return nc


_NC = None


def _get_nc():
    global _NC
    if _NC is None:
        _NC = build_nc()
    return _NC


def _perm():
    """Column permutation for Wq/Wk: new col (j*256 + p*128 + r) holds
    original inner dim h*64 + t with h = 4j + r//32, t = 32p + r%32."""
    perm = np.empty(INNER, dtype=np.int64)
    for j in range(2):
        for p in range(2):
            for r in range(128):
                h = 4 * j + r // 32
                t = 32 * p + r % 32
                perm[j * 256 + p * 128 + r] = h * DH + t
    return perm


def kernel(x, Wq, Wk, Wv, Wo, bo):
    x = np.asarray(x, dtype=np.float32)
    Wq = np.asarray(Wq, dtype=np.float32)
    Wk = np.asarray(Wk, dtype=np.float32)
    Wv = np.asarray(Wv, dtype=np.float32)
    Wo = np.asarray(Wo, dtype=np.float32)
    bo = np.asarray(bo, dtype=np.float32)

    B = x.shape[0]
    perm = _perm()
    nc = _get_nc()
    in_maps = []
    for c in range(8):
        b, hh = c // 2, c % 2
        sl = slice(hh * INNER, hh * INNER + INNER)
        in_maps.append({
            "xT": np.ascontiguousarray(x[b].T),
            "wq": np.ascontiguousarray(Wq[:, sl][:, perm]),
            "wk": np.ascontiguousarray(Wk[:, sl][:, perm]),
            "wv": np.ascontiguousarray(Wv[:, sl]),
            "wo": np.ascontiguousarray(Wo[sl, :].astype(np.float16)),
        })
    res = run_bass_kernel_spmd(nc, in_maps, core_ids=list(range(8)))
    out = np.empty((B, N, DIM), dtype=np.float32)
    for b in range(B):
        out[b] = res.results[2 * b]["out"] + res.results[2 * b + 1]["out"] + bo
    return out


# revision 13
# speedup vs baseline: 1.2027x; 1.1670x over previous
"""Multi-head attention (B=4, N=2048, DIM=1024, H=16, DH=64) on 8 trn2 cores.

Sharding: data-parallel over batch (4) x tensor-parallel over heads (2 groups
of 8). Each core computes q/k/v projections for its 8 heads, attention, and a
partial output projection; the host sums the two partials per batch and adds
the bias.

Design (per core):
  - Host sends x pre-transposed (xT [DIM, N] f32) - no on-chip transposes.
  - Projections in f32r (exact): psum = q/k/v unscaled.
  - q/k evacuate to fp8e4 at scale 16 in a [32-row x 2-slot] per-head layout
    (host permutes Wq/Wk columns) so the score matmuls run in fp8 DoubleRow
    mode (2 k-tiles per instruction, 0.5 cyc/row): scores_psum = 256*q.k.
  - v evacuates to fp8 hi + fp8 lo residual (scalar_tensor_tensor), giving
    ~fp16 effective precision for attn@v while keeping DoubleRow speed.
  - exp: split across ScalarE (native Exp -> fp8, scale 1/2048 folded) and
    DVE (Schraudolph: i8 = round(psum*A + B) written to an int8 view of the
    fp8 tile - the bitcast i8 IS the fp8 exp approximation). GPSIMD cannot
    read PSUM on trn2 so it only issues the x DMAs.
  - attn@v flipped: out[q, dh] = expT.T @ v_aug per query-tile, DoubleRow
    over kt pairs, one psum accumulation group per bank (2KB zero region);
    ones column (0.25) in v_hi gives the softmax denominator per PARTITION
    (query): normalization is reciprocal + stride-0-broadcast multiply
    writing fp16 attn.
  - attn fp16 is PE-transposed to attnT (psum fp16), then the output
    projection runs fp16 x fp16 (Wo fp16 from host), psum = 16*out, evac
    scales 1/16.
  - Schedule: short phase A (k + q(qb0) projections, DMA-paced on two
    queues), then one exp-bound main loop over (qb, h, g) units. v and
    q(qb1-3) projections and the previous qb's transposes + Wo matmuls are
    queued as filler thunks the PE consumes between score matmuls.
"""
import numpy as np
import ml_dtypes

import concourse.bass as bass
import concourse.mybir as mybir
import concourse.tile as tile
from concourse import bacc
from concourse.bass_utils import run_bass_kernel_spmd
from concourse.masks import make_identity

f32 = mybir.dt.float32
f32r = mybir.dt.float32r
f16 = mybir.dt.float16
f8 = mybir.dt.float8e4
i8 = mybir.dt.int8
AF = mybir.ActivationFunctionType
ALU = mybir.AluOpType
DR = mybir.MatmulPerfMode.DoubleRow
FP8 = ml_dtypes.float8_e4m3

N = 2048          # tokens
DIM = 1024        # model dim
NHL = 8           # heads per core
DH = 64           # head dim
INNER = NHL * DH  # 512 per-core inner dim
SCALE = DH ** -0.5
NDC = DIM // 128  # 8 dim chunks
SQ = 16.0         # q/k/v fp8 scale
EXP_A = 8 * 1.4426950408889634 * SCALE / (SQ * SQ)   # i8 slope
EXP_B = 8 * (7.0 - 0.0579)                           # i8 bias
QB = 512
NQB = N // QB
NG = 8            # kt-pair groups per query block

EXP_PAT = (0, 1, 0, 1, 0, 1, 0, 1)   # 0=ScalarE, 1=DVE
AV_LAG = 5
FILL_PER_G = 1


def build_nc():
    nc = bacc.Bacc(None, target_bir_lowering=False)

    xT_d = nc.dram_tensor("xT", [DIM, N], f32r, kind="ExternalInput")
    wq_d = nc.dram_tensor("wq", [DIM, INNER], f32r, kind="ExternalInput")
    wk_d = nc.dram_tensor("wk", [DIM, INNER], f32r, kind="ExternalInput")
    wv_d = nc.dram_tensor("wv", [DIM, INNER], f32r, kind="ExternalInput")
    wo_d = nc.dram_tensor("wo", [INNER, DIM], f16, kind="ExternalInput")
    out_d = nc.dram_tensor("out", [N, DIM], f32, kind="ExternalOutput")

    xT_v = xT_d.rearrange("(c k) n -> k c n", k=128)   # [128, 8, N]
    wq_v = wq_d.rearrange("(c k) n -> k c n", k=128)   # [128, 8, INNER]
    wk_v = wk_d.rearrange("(c k) n -> k c n", k=128)
    wv_v = wv_d.rearrange("(c k) n -> k c n", k=128)
    wo_v = wo_d.rearrange("(c k) n -> k c n", k=128)   # [128, 4, DIM]

    with tile.TileContext(nc) as tc:
        with (
            tc.tile_pool(name="consts", bufs=1) as consts,
            tc.tile_pool(name="xin", bufs=1) as xin,
            tc.tile_pool(name="wp", bufs=1) as wp,
            tc.tile_pool(name="wop", bufs=1) as wop,
            tc.tile_pool(name="qk", bufs=1) as qkp,
            tc.tile_pool(name="vp", bufs=1) as vp,
            tc.tile_pool(name="expp", bufs=AV_LAG + 3) as expp,
            tc.tile_pool(name="anat", bufs=2) as anatp,
            tc.tile_pool(name="atp", bufs=2) as atp,
            tc.tile_pool(name="outp", bufs=2) as outp,
            tc.tile_pool(name="recipp", bufs=3) as recipp,
            tc.tile_pool(name="ps_s", bufs=3, space="PSUM") as ps_s,
            tc.tile_pool(name="ps_av", bufs=1, space="PSUM") as ps_av,
            tc.tile_pool(name="ps_f", bufs=1, space="PSUM") as ps_f,
        ):
            ident = consts.tile([128, 128], f16)
            make_identity(nc, ident)

            wk_sb = wp.tile([128, NDC, INNER], f32r)
            wv_sb = wp.tile([128, NDC, INNER], f32r)
            wq_sb = wp.tile([128, NDC, INNER], f32r)
            wo_sb = wop.tile([128, 4, DIM], f16)
            xts = [xin.tile([128, 2, N], f32r, name=f"xt{j}", tag=f"xt{j}")
                   for j in range(4)]
            # weights on the SP DMA queue; x token-pieces on the Pool queue
            nc.sync.dma_start(wk_sb, wk_v[:, :, :])
            for tb in range(4):
                t0 = tb * 512
                for j in range(4):
                    nc.gpsimd.dma_start(
                        xts[j][:, :, t0:t0 + 512],
                        xT_v[:, 2 * j:2 * j + 2, t0:t0 + 512])
            nc.sync.dma_start(wq_sb, wq_v[:, :, :])
            nc.sync.dma_start(wv_sb, wv_v[:, :, :])
            nc.sync.dma_start(wo_sb, wo_v[:, :, :])

            # fp8 q/k in 32x2 DoubleRow layout: [128, j, pair, tok]
            kT2 = qkp.tile([128, 2, 2, N], f8)
            qT2 = qkp.tile([128, 2, 2, N], f8)
            # v hi/lo: [key%128, ktpair, i, h, DH+1]; ones col
            v_hi = vp.tile([128, NG, 2, NHL, DH + 1], f8)
            v_lo = vp.tile([128, NG, 2, NHL, DH + 1], f8)
            nc.vector.memset(v_hi[:, :, :, :, DH], 1.0)
            nc.vector.memset(v_lo[:, :, :, :, DH], 0.0)

            # ---------------- projection emitters ----------------
            evac_rr = [0]

            def qk_chain(w_sb, dest, tb, j):
                t0 = tb * 512
                ps = ps_s.tile([128, 2, 512], f32, name=f"pp{tb}{j}",
                               tag="s")
                for p in range(2):
                    m0 = (2 * j + p) * 128
                    for dc in range(NDC):
                        nc.tensor.matmul(
                            ps[:, p, :],
                            w_sb[:, dc, m0:m0 + 128],
                            xts[dc // 2][:, dc % 2, t0:t0 + 512],
                            start=(dc == 0), stop=(dc == NDC - 1))
                if evac_rr[0] % 2 == 0:
                    nc.vector.tensor_scalar(dest[:, j, :, t0:t0 + 512],
                                            ps, SQ, None, ALU.mult)
                else:
                    nc.scalar.activation(out=dest[:, j, :, t0:t0 + 512],
                                         in_=ps, func=AF.Copy, scale=SQ)
                evac_rr[0] += 1

            def v_chain(g):
                # one kt-pair g -> v_hi/v_lo tile pair
                ps = ps_s.tile([128, 2, 512], f32, name=f"pv{g}", tag="s")
                for i in range(2):
                    kt = 2 * g + i
                    for dc in range(NDC):
                        nc.tensor.matmul(
                            ps[:, i, :],
                            xts[dc // 2][:, dc % 2,
                                         kt * 128:kt * 128 + 128],
                            wv_sb[:, dc, :],
                            start=(dc == 0), stop=(dc == NDC - 1))
                psv = ps.rearrange("p two (h d) -> p two h d", h=NHL)
                nc.scalar.activation(out=v_hi[:, g, :, :, 0:DH],
                                     in_=psv, func=AF.Copy, scale=SQ)
                nc.vector.scalar_tensor_tensor(
                    v_lo[:, g, :, :, 0:DH], psv, SQ,
                    v_hi[:, g, :, :, 0:DH], ALU.mult, ALU.subtract)

            # ---------------- phase A: k (all) + q(qb0) ----------------
            for tb in range(4):
                for j in range(2):
                    qk_chain(wk_sb, kT2, tb, j)
            for j in range(2):
                qk_chain(wq_sb, qT2, 0, j)

            # filler: v chains first (needed by attn@v), then q(qb1-3)
            filler = []
            for g in range(NG):
                filler.append(lambda g=g: v_chain(g))
            for tb in range(1, 4):
                for j in range(2):
                    filler.append(
                        lambda tb=tb, j=j: qk_chain(wq_sb, qT2, tb, j))

            # ---------------- attention main loop ----------------
            av_q = []  # (av, et, g, h, anat, is_last)

            def deq():
                av, et, g, h, anat, is_last = av_q.pop(0)
                # one accumulation group for the whole bank: start only on
                # the very first matmul, stop on the very last (psum zero
                # regions are 2KB, so per-qt groups would wipe each other)
                for qt in range(4):
                    lhs = et[:, :, qt * 128:qt * 128 + 128]
                    nc.tensor.matmul(av[:, qt, :], lhs, v_hi[:, g, :, h, :],
                                     start=(g == 0 and qt == 0), stop=False,
                                     perf_mode=DR, skip_group_check=True)
                    nc.tensor.matmul(av[:, qt, :], lhs, v_lo[:, g, :, h, :],
                                     start=False,
                                     stop=(g == NG - 1 and qt == 3),
                                     perf_mode=DR, skip_group_check=True)
                if is_last:
                    emit_norm(av, anat, h)

            def emit_norm(av, anat, h):
                recip = recipp.tile([128, 4, 1], f32, name=f"rc{h}",
                                    tag="rc")
                nc.vector.reciprocal(recip, av[:, :, DH:DH + 1])
                nc.vector.tensor_tensor(
                    anat[:, :, h, :], av[:, :, 0:DH],
                    recip.broadcast_to([128, 4, DH]), ALU.mult)

            def wo_fillers(anat, qb):
                """Filler thunks for one qb: 4 transpose quads + 4 Wo
                chains (each with evac + out DMA)."""
                attnT = atp.tile([128, 4, QB], f16, name=f"aT{qb}",
                                 tag="aT")

                def transp(qt):
                    ptr = ps_f.tile([128, 4, 128], f16, name=f"ptr{qt}",
                                    tag="f")
                    av_in = anat[:, qt, :, :].rearrange("p h d -> p (h d)")
                    for c in range(4):
                        nc.tensor.transpose(
                            ptr[:, c, :], av_in[:, c * 128:c * 128 + 128],
                            ident)
                    nc.scalar.activation(
                        out=attnT[:, :, qt * 128:qt * 128 + 128], in_=ptr,
                        func=AF.Copy, scale=1.0)

                def wo_chain(qs):
                    osb = outp.tile([128, DIM], f32, name=f"osb{qs}",
                                    tag="osb")
                    for d in range(2):
                        psf = ps_f.tile([128, 512], f32, name=f"psf{qs}{d}",
                                        tag="f")
                        for c in range(4):
                            nc.tensor.matmul(
                                psf,
                                attnT[:, c, qs * 128:qs * 128 + 128],
                                wo_sb[:, c, d * 512:d * 512 + 512],
                                start=(c == 0), stop=(c == 3))
                        nc.scalar.activation(out=osb[:, d * 512:d * 512 + 512],
                                             in_=psf, func=AF.Copy,
                                             scale=1.0 / SQ)
                    r0 = qb * QB + qs * 128
                    nc.sync.dma_start(out_d[r0:r0 + 128, :], osb)

                return ([(lambda qt=qt: transp(qt)) for qt in range(4)]
                        + [(lambda qs=qs: wo_chain(qs)) for qs in range(4)])

            for qb in range(NQB):
                anat = anatp.tile([128, 4, NHL, DH], f16, name=f"an{qb}",
                                  tag="an")
                for h in range(NHL):
                    j, po = h // 4, 32 * (h % 4)
                    av = ps_av.tile([128, 4, DH + 1], f32, name=f"av{h}",
                                    tag="av")
                    for g in range(NG):
                        pss = ps_s.tile([128, 2, QB], f32, name=f"pss{g}",
                                        tag="s")
                        for i in range(2):
                            kt = 2 * g + i
                            nc.tensor.matmul(
                                pss[:, i, :],
                                kT2[po:po + 32, j, :,
                                    kt * 128:kt * 128 + 128],
                                qT2[po:po + 32, j, :, qb * QB:(qb + 1) * QB],
                                start=True, stop=True, perf_mode=DR,
                                tile_position=(po, 0))
                        et = expp.tile([128, 2, QB], f8, name=f"et{h}{g}",
                                       tag="et")
                        if EXP_PAT[(h + qb + g) % 8] == 0:
                            nc.scalar.activation(out=et, in_=pss,
                                                 func=AF.Exp,
                                                 scale=SCALE / (SQ * SQ))
                        else:
                            nc.vector.tensor_scalar(et.bitcast(i8), pss,
                                                    EXP_A, EXP_B,
                                                    ALU.mult, ALU.add)
                        av_q.append((av, et, g, h, anat, g == NG - 1))
                        if len(av_q) > AV_LAG:
                            deq()
                        for _ in range(FILL_PER_G):
                            if filler:
                                filler.pop(0)()
                while av_q:
                    deq()
                filler.extend(wo_fillers(anat, qb))
            while filler:
                filler.pop(0)()

    nc.compile()
    return nc


_NC = None


def _get_nc():
    global _NC
    if _NC is None:
        _NC = build_nc()
    return _NC


def _perm():
    """Column permutation for Wq/Wk: new col (j*256 + p*128 + r) holds
    original inner dim h*64 + t with h = 4j + r//32, t = 32p + r%32."""
    perm = np.empty(INNER, dtype=np.int64)
    for j in range(2):
        for p in range(2):
            for r in range(128):
                h = 4 * j + r // 32
                t = 32 * p + r % 32
                perm[j * 256 + p * 128 + r] = h * DH + t
    return perm


def kernel(x, Wq, Wk, Wv, Wo, bo):
    x = np.asarray(x, dtype=np.float32)
    Wq = np.asarray(Wq, dtype=np.float32)
    Wk = np.asarray(Wk, dtype=np.float32)
    Wv = np.asarray(Wv, dtype=np.float32)
    Wo = np.asarray(Wo, dtype=np.float32)
    bo = np.asarray(bo, dtype=np.float32)

    B = x.shape[0]
    perm = _perm()
    nc = _get_nc()
    in_maps = []
    for c in range(8):
        b, hh = c // 2, c % 2
        sl = slice(hh * INNER, hh * INNER + INNER)
        in_maps.append({
            "xT": np.ascontiguousarray(x[b].T),
            "wq": np.ascontiguousarray(Wq[:, sl][:, perm]),
            "wk": np.ascontiguousarray(Wk[:, sl][:, perm]),
            "wv": np.ascontiguousarray(Wv[:, sl]),
            "wo": np.ascontiguousarray(Wo[sl, :].astype(np.float16)),
        })
    res = run_bass_kernel_spmd(nc, in_maps, core_ids=list(range(8)))
    out = np.empty((B, N, DIM), dtype=np.float32)
    for b in range(B):
        out[b] = res.results[2 * b]["out"] + res.results[2 * b + 1]["out"] + bo
    return out
